# revision 20
# baseline (speedup 1.0000x reference)
"""Trainium2 Bass kernel for AllegroScalarOutputHead (segment_reduce).

Strategy (8 NeuronCores, SPMD, no collectives, no indirect DMA):
  - Graphs 4k..4k+3 -> core k (batch is sorted => contiguous node range).
    Edges go to the core owning their TARGET node's graph.
  - All index math is done on the host (free): per-edge coefficient
    c_e = pair_scales[zs*101+zt] * atom_scales[zt] folded into a per-graph
    one-hot coefficient table c4e[p, g, j]; per-node ascale folded into
    c4n[p, g, j].  Constant shift/bias terms are summed on the host.
  - Device does only dense streaming math, organized as a flat list of
    <=1536-col supertile units (edge and node), software-pipelined on PE
    (mm1 of unit i issues before mm2 of unit i-1) so the PE never blocks
    on the ACT engine's silu of the same unit.
  - The ACT (scalar) engine is the bottleneck (~1 col/ns silu, dtype
    independent, ~300ns fixed cost per ACTIVATE), so the schedule is
    built around keeping it 100% busy:
    * tiny first ramp block (512 cols) + split weight DMA (edge weights
      are only 33KB) -> first silu at ~4us instead of ~10us
    * ALL DMAs ride the sync ring (the scalar queue is left to ACTIVATEs
      only; its ACT_TABLE_LOAD at t=0 would also serialize any DMA issue
      behind it); ring FIFO order = emission order gives deterministic
      delivery ordering
    * big tables (c4e, nTa, nTb, c4n) are emitted between LATE steady
      edge blocks, where the delivery-vs-consumption slack has built up
    * the c4e-dependent per-block reductions are DEFERRED until c4e has
      safely landed, so the strict-FIFO DVE queue never holds up the
      pe_sb PSUM drains (which gate PE's single pe PSUM bank, which
      gates mm1, which gates ACT)
  - Feature streams are fp16 (halves HBM traffic; verified error budget
    vs the 2e-2 gate on this exact problem instance).
"""

import numpy as np

NCORES = 8
N_NODES = 50000
NUM_GRAPHS = 32
NZ = 101            # atomic number table entries (0..100)
D_EDGE = 128
D_NODE = 256
EDGE_BLOCK = 6144   # steady-state edge columns per DMA block (12KB packets)
SUPER = 1536        # act supertile (3 PSUM banks)
ALIGN = 512         # ET alignment (matches mm1 chunk size)

RAMP = (1024, 2560, 2560, 3072, 3072, 3072, 3072)  # escalating ramp (18432)
# table DMAs are emitted just after these blocks' first units (ring order):
TRIG_C4E_BLK = 9
TRIG_NTA_BLK = 15
TRIG_NTB_BLK = 16
TRIG_C4N_BLK = 17
TT_SAFE_BLK = 10      # block reductions flush from this block's first unit

_CACHE = {}


def _edge_blocks(ET):
    """(start, ncols) DMA blocks: escalating ramp then full-width blocks."""
    blocks = []
    pos = 0
    for n in RAMP:
        blocks.append((pos, n))
        pos += n
    while ET - pos >= EDGE_BLOCK:
        blocks.append((pos, EDGE_BLOCK))
        pos += EDGE_BLOCK
    if ET - pos:
        blocks.append((pos, ET - pos))
        pos += ET - pos
    assert pos == ET
    return blocks


def _node_unit_chunks(NT):
    """Greedy (pos, ncols<=512) chunk groups, each group <= SUPER wide."""
    chunks = []
    pos = 0
    while pos < NT:
        c = min(512, NT - pos)
        chunks.append((pos, c))
        pos += c
    units = []
    cur, w = [], 0
    for pos, c in chunks:
        if w + c > SUPER:
            units.append(cur)
            cur, w = [], 0
        cur.append((pos, c))
        w += c
    if cur:
        units.append(cur)
    return units


def _build(ET, NT):
    import concourse.tile as tile
    from concourse import bacc, mybir
    from contextlib import ExitStack

    f32 = mybir.dt.float32
    f16 = mybir.dt.float16
    AF = mybir.ActivationFunctionType
    OP = mybir.AluOpType

    EC = ET // 128
    NTC = NT // 128
    blocks = _edge_blocks(ET)
    NBLK = len(blocks)
    node_units_chunks = _node_unit_chunks(NT)

    nc = bacc.Bacc("TRN2", debug=False, num_devices=NCORES)

    # eT is prefixed with the 129 We columns (W1e | W2e) so the first
    # ramp DMA delivers weights and data in one shot
    eT_d = nc.declare_dram_parameter("eT", [128, 129 + ET], f16, isOutput=False)
    c4e_d = nc.declare_dram_parameter("c4e", [128, 4 * EC], f32, isOutput=False)
    nTa_d = nc.declare_dram_parameter("nTa", [128, NT], f16, isOutput=False)
    nTb_d = nc.declare_dram_parameter("nTb", [128, NT], f16, isOutput=False)
    c4n_d = nc.declare_dram_parameter("c4n", [128, 4 * NTC], f32, isOutput=False)
    # node weights: W1n q00 q01 q10 q11 | W2n  (fp16, 514 cols)
    Wn_d = nc.declare_dram_parameter("Wn", [128, 514], f16, isOutput=False)
    # packed biases: b1e | b1n0 | b1n1  (fp32)
    bp_d = nc.declare_dram_parameter("bp", [128, 3], f32, isOutput=False)
    out1_d = nc.declare_dram_parameter("out1", [128, 4], f32, isOutput=True)
    LBC = blocks[-1][1] // 128
    out2_d = nc.declare_dram_parameter("out2", [128, LBC], f32, isOutput=True)
    out3_d = nc.declare_dram_parameter("out3", [128, 4], f32, isOutput=True)

    with tile.TileContext(nc) as tc, ExitStack() as ctx:
        const = ctx.enter_context(tc.tile_pool(name="const", bufs=1))
        edgep = ctx.enter_context(tc.tile_pool(name="edgep", bufs=5))
        hep = ctx.enter_context(tc.tile_pool(name="hep", bufs=5))
        dvep = ctx.enter_context(tc.tile_pool(name="dvep", bufs=3))
        ps_big = ctx.enter_context(tc.tile_pool(name="ps_big", bufs=2, space="PSUM"))
        ps_pe = ctx.enter_context(tc.tile_pool(name="ps_pe", bufs=1, space="PSUM"))
        ps_acc = ctx.enter_context(tc.tile_pool(name="ps_acc", bufs=1, space="PSUM"))

        # preload the Silu ACT table at t=0 with a dummy 1-col activation so
        # the 2.6us table load overlaps the startup DMAs
        ones0 = const.tile([128, 1], f32)
        nc.vector.memset(ones0[:], 1.0)
        dummy = const.tile([128, 1], f32)
        nc.scalar.activation(dummy[:], ones0[:], AF.Silu)

        # ---- PE warm-up / heartbeat setup ----------------------------------
        # the PE starts HAM-throttled (K=4/8, 1.2 GHz), re-warms only after
        # ~3.4us of sustained activity, and re-throttles whenever it idles
        # too long.  Warm it with garbage MMs during the DMA-latency shadow
        # and keep it warm with one tiny heartbeat MM per unit, all writing
        # spare columns of the persistent pa PSUM bank from a resident
        # (DMA-independent) source tile.
        warm = const.tile([128, 640], f16, name="warmsrc")
        nc.vector.memset(warm[:], 1.0)

        # ---- startup DMAs ---------------------------------------------------
        # ALL DMAs ride the single sync ring: the two HWDGE rings do NOT
        # share SDMA bandwidth fairly (one can starve the other for
        # microseconds), so a strict single-FIFO whose escalating sizes
        # match the serial landing cadence is both faster and predictable
        ramp_xe = []
        r0t = const.tile([128, 129 + RAMP[0]], f16, tag="rampxe0", name="rampxe0")
        nc.sync.dma_start(r0t[:], eT_d.ap()[:, 0:129 + RAMP[0]])
        ramp_xe.append(r0t[:, 129:129 + RAMP[0]])
        We = r0t[:, 0:129]
        bp = const.tile([128, 3], f32)
        nc.sync.dma_start(bp[:], bp_d.ap())
        for r in range(1, len(RAMP)):
            t = const.tile([128, RAMP[r]], f16, tag=f"rampxe{r}", name=f"rampxe{r}")
            pos0 = sum(RAMP[:r])
            nc.sync.dma_start(t[:], eT_d.ap()[:, 129 + pos0:129 + pos0 + RAMP[r]])
            ramp_xe.append(t)
        Wn = const.tile([128, 514], f16)
        nc.sync.dma_start(Wn[:], Wn_d.ap())

        W1e = We[:, 0:128]
        W2e = We[:, 128:129]
        W1n = [Wn[:, q * 128:(q + 1) * 128] for q in range(4)]  # kb*2+db
        W2n = Wn[:, 512:514]
        b1e = bp[:, 0:1]
        b1n = bp[:, 1:3]

        c4e_s = const.tile([128, 4, EC], f32)
        c4n_s = const.tile([128, 4, NTC], f32)
        nTa_s = const.tile([128, NT], f16)
        nTb_s = const.tile([128, NT], f16)
        ones = const.tile([128, 1], f32)
        nc.vector.memset(ones[:], 1.0)

        redsb = const.tile([128, 4, NBLK], f32)
        pesb_all = const.tile([128, NBLK, EDGE_BLOCK // 128], f32)
        pa_all = const.tile([128, NTC], f32)
        pa_ps = ps_acc.tile([128, 512], f32, tag="pa")

        def trig_c4e():
            nc.sync.dma_start(
                c4e_s[:], c4e_d.ap().rearrange("p (g j) -> p g j", g=4))

        def trig_nta():
            nc.sync.dma_start(nTa_s[:], nTa_d.ap())

        def trig_ntb():
            nc.sync.dma_start(nTb_s[:], nTb_d.ap())

        def trig_c4n():
            nc.sync.dma_start(
                c4n_s[:], c4n_d.ap().rearrange("p (g j) -> p g j", g=4))

        # ---- supertile units -----------------------------------------------
        class EdgeState:
            xe = None
            pe_ps = None

        est = [EdgeState() for _ in range(NBLK)]
        deferred = []  # (flush_at_emit_idx, fn): c4e-dependent block reduces

        def make_block_reduce(b):
            pos, ncols = blocks[b]
            cpb = ncols // 128

            def tt():
                jc = slice(pos // 128, pos // 128 + cpb)
                tmpb = dvep.tile(
                    [128, 4, EDGE_BLOCK // 128], f32, tag="tmpb")
                nc.vector.tensor_tensor(
                    tmpb[:, :, 0:cpb], c4e_s[:, :, jc],
                    pesb_all[:, b, 0:cpb].unsqueeze(1).broadcast_to(
                        [128, 4, cpb]),
                    OP.mult,
                )
                nc.vector.tensor_reduce(
                    redsb[:, :, b:b + 1], tmpb[:, :, 0:cpb],
                    mybir.AxisListType.X, OP.add,
                )

            return tt

        def make_edge_unit(b, h):
            pos, ncols = blocks[b]
            hc = min(SUPER, ncols - h * SUPER)
            n512 = -(-hc // 512)
            first = h == 0
            last = (h + 1) * SUPER >= ncols
            cpb = ncols // 128

            def mm1():
                if first:
                    if b < len(RAMP):
                        est[b].xe = ramp_xe[b]
                    else:
                        est[b].xe = edgep.tile(
                            [128, EDGE_BLOCK], f16, tag="xe", name="xe")
                        nc.sync.dma_start(
                            est[b].xe[:, 0:ncols],
                            eT_d.ap()[:, 129 + pos:129 + pos + ncols])
                    est[b].pe_ps = ps_pe.tile(
                        [128, EDGE_BLOCK // 128], f32, tag="pe", name="pe_ps")
                ps = ps_big.tile([128, SUPER], f32, tag="mm1")
                for q in range(n512):
                    qc = min(512, hc - q * 512)
                    nc.tensor.matmul(
                        ps[:, q * 512:q * 512 + qc], W1e,
                        est[b].xe[:, h * SUPER + q * 512:h * SUPER + q * 512 + qc],
                        start=True, stop=True,
                    )
                return ps

            def act(ps):
                he = hep.tile([128, SUPER], f16, tag="he")
                nc.scalar.activation(
                    he[:, 0:hc], ps[:, 0:hc], AF.Silu, bias=b1e)
                return he

            def mm2(he, emit_idx, tt_safe):
                for t in range(hc // 128):
                    col = h * (SUPER // 128) + t
                    nc.tensor.matmul(
                        est[b].pe_ps[:, col:col + 1],
                        he[:, t * 128:(t + 1) * 128], W2e,
                        start=True, stop=True,
                    )
                if last:
                    # prompt PSUM drain (frees the single pe bank for the
                    # next block); the c4e-dependent reduce is deferred so
                    # it never blocks this copy in the FIFO DVE queue
                    nc.vector.tensor_copy(
                        pesb_all[:, b, 0:cpb], est[b].pe_ps[:, 0:cpb])
                    if b < NBLK - 1:
                        deferred.append(
                            (max(emit_idx + 1, tt_safe), make_block_reduce(b)))
                    else:
                        nc.sync.dma_start(
                            out2_d.ap(), pesb_all[:, b, 0:LBC])

            return mm1, act, mm2

        def make_node_pair(chunks):
            """Two act units (K-halves db=0/1) covering the given chunk
            list.  The second unit drains both halves' mm2 back-to-back so
            each pa column's PSUM accumulation group is contiguous."""
            width = sum(c for _, c in chunks)
            j0 = chunks[0][0]
            stash = {}

            def mk(db):
                def mm1():
                    ps = ps_big.tile([128, SUPER], f32, tag="mm1")
                    off = 0
                    for pos, c in chunks:
                        nc.tensor.matmul(
                            ps[:, off:off + c], W1n[0 * 2 + db],
                            nTa_s[:, pos:pos + c],
                            start=True, stop=False,
                        )
                        nc.tensor.matmul(
                            ps[:, off:off + c], W1n[1 * 2 + db],
                            nTb_s[:, pos:pos + c],
                            start=False, stop=True,
                        )
                        off += c
                    return ps

                def act(ps):
                    he = hep.tile([128, SUPER], f16, tag="he")
                    nc.scalar.activation(
                        he[:, 0:width], ps[:, 0:width], AF.Silu,
                        bias=b1n[:, db:db + 1])
                    return he

                def mm2(he, emit_idx, tt_safe):
                    if db == 0:
                        stash["he0"] = he
                        return
                    for t in range(width // 128):
                        col = j0 // 128 + t
                        nc.tensor.matmul(
                            pa_ps[:, col:col + 1],
                            stash["he0"][:, t * 128:(t + 1) * 128],
                            W2n[:, 0:1], start=True, stop=False,
                        )
                        nc.tensor.matmul(
                            pa_ps[:, col:col + 1],
                            he[:, t * 128:(t + 1) * 128],
                            W2n[:, 1:2], start=False, stop=True,
                        )

                return mm1, act, mm2

            return mk(0), mk(1)

        # edge units tagged with their block index
        units = []  # (block_or_None, is_last_node, (mm1, act, mm2))
        for b in range(NBLK):
            _, ncols = blocks[b]
            for h in range(-(-ncols // SUPER)):
                units.append((b, False, make_edge_unit(b, h)))

        def first_unit_idx(blk):
            for i, (b, _, _) in enumerate(units):
                if b == blk:
                    return i
            return len(units) - 1

        # node pairs appended after all edge units
        edge_last_pos = len(units) - 1
        node_pairs = [make_node_pair(ch) for ch in node_units_chunks]
        for pi, (ua, ub) in enumerate(node_pairs):
            last = pi == len(node_pairs) - 1
            units.append((None, False, ua))
            units.append((None, last, ub))
        node_last_pos = len(units) - 1

        triggers = {}
        for blk, fn in ((TRIG_C4E_BLK, trig_c4e), (TRIG_NTA_BLK, trig_nta),
                        (TRIG_NTB_BLK, trig_ntb), (TRIG_C4N_BLK, trig_c4n)):
            idx = first_unit_idx(min(blk, NBLK - 1))
            while idx in triggers:
                idx += 1
            triggers[idx] = fn
        tt_safe = first_unit_idx(min(TT_SAFE_BLK, NBLK - 1)) + 2

        # node-side final reduction, emitted right after the last node unit
        redn = const.tile([128, 4], f32)

        def node_tail():
            nc.vector.tensor_copy(pa_all[:], pa_ps[:, 0:NTC])
            tmpn = const.tile([128, 4, NTC], f32)
            nc.vector.tensor_tensor(
                tmpn[:], c4n_s[:],
                pa_all[:].unsqueeze(1).broadcast_to([128, 4, NTC]), OP.mult,
            )
            nc.vector.tensor_reduce(
                redn[:].unsqueeze(2), tmpn[:], mybir.AxisListType.X, OP.add,
            )

        # ---- software-pipelined emission -----------------------------------
        red = const.tile([128, 4], f32)  # per-partition per-graph partials
        pending = None  # (mm2, he, emit_idx) of previous unit
        node_tail_done = False
        red_done = False

        def emit_out1():
            # edge blocks except the final one (whose 1-unit sliver is
            # reduced on the host from out2); node partials go via out3
            nc.vector.tensor_reduce(
                red[:].unsqueeze(2), redsb[:, :, 0:NBLK - 1],
                mybir.AxisListType.X, OP.add,
            )
            nc.sync.dma_start(out1_d.ap(), red[:])

        for i, (_, _, (mm1, act, mm2)) in enumerate(units):
            ps = mm1()
            he = act(ps)
            if pending is not None:
                pending[0](pending[1], pending[2], tt_safe)
            pending = (mm2, he, i)
            if i in triggers:
                triggers[i]()
            while deferred and deferred[0][0] <= i:
                deferred.pop(0)[1]()
            if i == edge_last_pos and len(deferred) == 0:
                emit_out1()
                red_done = True
        pending[0](pending[1], pending[2], tt_safe)
        for _, fn in deferred:
            fn()
        if not red_done:
            emit_out1()
        node_tail()
        nc.sync.dma_start(out3_d.ap(), redn[:])

    nc.compile()
    return nc


def _shard(inputs):
    node_feats = np.asarray(inputs["node_feats"], dtype=np.float32)
    edge_feats = np.asarray(inputs["edge_feats"], dtype=np.float32)
    Z = np.asarray(inputs["atomic_numbers"], dtype=np.int64)
    idx_s = np.asarray(inputs["idx_s"], dtype=np.int64)
    idx_t = np.asarray(inputs["idx_t"], dtype=np.int64)
    batch = np.asarray(inputs["batch"], dtype=np.int64)
    asc = np.asarray(inputs["atom_scales"], np.float32)[:, 0]
    ash = np.asarray(inputs["atom_shifts"], np.float32)[:, 0]
    pscale = np.asarray(inputs["pair_scales"], np.float32)[:, 0]
    b2e = float(np.asarray(inputs["b2e"], np.float32).reshape(-1)[0])
    b2n = float(np.asarray(inputs["b2n"], np.float32).reshape(-1)[0])

    bounds = np.searchsorted(batch, np.arange(NUM_GRAPHS + 1))
    g_t = batch[idx_t]
    core_of_edge = np.minimum(g_t // 4, NCORES - 1)
    c_e = (pscale[Z[idx_s] * NZ + Z[idx_t]] * asc[Z[idx_t]]).astype(np.float32)

    e_counts = np.bincount(core_of_edge, minlength=NCORES)
    ET = int(-(-e_counts.max() // ALIGN) * ALIGN)
    # keep at least one full-width block after the ramp
    ET = max(ET, sum(RAMP) + EDGE_BLOCK)
    n_counts = bounds[4 * np.arange(NCORES) + 4] - bounds[4 * np.arange(NCORES)]
    NT = int(-(-n_counts.max() // 128) * 128)

    # constant (device-independent) per-graph terms
    asc_n = asc[Z]
    host_add = np.zeros(NUM_GRAPHS, np.float64)
    np.add.at(host_add, batch, (b2n * asc_n + ash[Z]).astype(np.float64))
    if b2e != 0.0:
        np.add.at(host_add, g_t, (b2e * c_e).astype(np.float64))

    order = np.argsort(core_of_edge, kind="stable")
    starts = np.concatenate([[0], np.cumsum(e_counts)])

    W1e = np.asarray(inputs["W1e"], np.float16)
    W1n = np.asarray(inputs["W1n"], np.float16)
    W2e = np.asarray(inputs["W2e"], np.float16).reshape(128, 1)
    W2n = np.asarray(inputs["W2n"], np.float16).reshape(2, 128).T
    We = np.zeros((128, 129), np.float16)
    We[:, 0:128] = W1e
    We[:, 128:129] = W2e
    Wn = np.zeros((128, 514), np.float16)
    for kb in range(2):
        for db in range(2):
            q = kb * 2 + db
            Wn[:, q * 128:(q + 1) * 128] = \
                W1n[kb * 128:(kb + 1) * 128, db * 128:(db + 1) * 128]
    Wn[:, 512:514] = W2n
    bp = np.zeros((128, 3), np.float32)
    bp[:, 0] = np.asarray(inputs["b1e"], np.float32)
    bp[:, 1:3] = np.asarray(inputs["b1n"], np.float32).reshape(2, 128).T

    blocks = _edge_blocks(ET)
    lb_pos, lb_ncols = blocks[-1]

    in_maps = []
    c4e_last = []
    for k in range(NCORES):
        sel = order[starts[k]:starts[k + 1]]
        E = sel.size
        eTk = np.zeros((128, 129 + ET), np.float16)
        eTk[:, 0:129] = We
        eTk[:, 129:129 + E] = edge_feats[sel].T
        c4e = np.zeros((ET, 4), np.float32)
        gl = g_t[sel] - 4 * k
        c4e[np.arange(E), gl] = c_e[sel]
        # last block's [ncols, 4] slice stays on the host: its per-edge
        # energies come back raw in out2 and are reduced here (keeps the
        # final-block reduction off the device's serial tail)
        c4e_last.append(c4e[lb_pos:lb_pos + lb_ncols].copy())
        # [ET,4] -> [128, 4, EC] with edge (j*128+p) at [p, :, j]
        c4e = np.ascontiguousarray(
            c4e.reshape(ET // 128, 128, 4).transpose(1, 2, 0)
        ).reshape(128, -1)

        n0 = int(bounds[4 * k])
        n1 = int(bounds[4 * k + 4])
        nn = n1 - n0
        nTk = np.zeros((256, NT), np.float16)
        nTk[:, :nn] = node_feats[n0:n1].T
        c4n = np.zeros((NT, 4), np.float32)
        c4n[np.arange(nn), batch[n0:n1] - 4 * k] = asc_n[n0:n1]
        c4n = np.ascontiguousarray(
            c4n.reshape(NT // 128, 128, 4).transpose(1, 2, 0)
        ).reshape(128, -1)

        in_maps.append({
            "eT": eTk, "c4e": c4e,
            "nTa": np.ascontiguousarray(nTk[:128]),
            "nTb": np.ascontiguousarray(nTk[128:]),
            "c4n": c4n,
            "Wn": Wn, "bp": bp,
        })
    return ET, NT, in_maps, host_add, c4e_last


LAST_RES = None
LAST_RES_NODE = None


def kernel(**inputs) -> np.ndarray:
    global LAST_RES
    from concourse.bass_utils import run_bass_kernel_spmd

    ET, NT, in_maps, host_add, c4e_last = _shard(inputs)
    key = (ET, NT)
    if key not in _CACHE:
        _CACHE[key] = _build(ET, NT)
    nc = _CACHE[key]

    res = run_bass_kernel_spmd(nc, in_maps, core_ids=list(range(NCORES)))
    LAST_RES = res
    Y = np.zeros(NUM_GRAPHS, np.float64)
    for k in range(NCORES):
        out1 = np.asarray(res.results[k]["out1"], np.float32)  # [128, 4]
        out2 = np.asarray(res.results[k]["out2"], np.float32)  # [128, LBC]
        out3 = np.asarray(res.results[k]["out3"], np.float32)  # [128, 4]
        Y[4 * k:4 * k + 4] += out1.sum(axis=0) + out3.sum(axis=0)
        # out2[p, col] = pe of edge (lb_pos + col*128 + p)
        pe_last = out2.T.reshape(-1)          # [ncols] in edge order
        Y[4 * k:4 * k + 4] += pe_last @ c4e_last[k]
    Y += host_add
    return Y.astype(np.float32)


# revision 21
# speedup vs baseline: 1.0345x; 1.0345x over previous
"""Trainium2 Bass kernel for AllegroScalarOutputHead (segment_reduce).

Strategy (8 NeuronCores, SPMD, no collectives, no indirect DMA):
  - Graphs 4k..4k+3 -> core k (batch is sorted => contiguous node range).
    Edges go to the core owning their TARGET node's graph.
  - All index math is done on the host (free): per-edge coefficient
    c_e = pair_scales[zs*101+zt] * atom_scales[zt] folded into a per-graph
    one-hot coefficient table c4e[p, g, j]; per-node ascale folded into
    c4n[p, g, j].  Constant shift/bias terms are summed on the host.
  - Device does only dense streaming math, organized as a flat list of
    <=1536-col supertile units (edge and node), software-pipelined on PE
    (mm1 of unit i issues before mm2 of unit i-1) so the PE never blocks
    on the ACT engine's silu of the same unit.
  - The ACT (scalar) engine is the bottleneck (~1 col/ns silu, dtype
    independent, ~300ns fixed cost per ACTIVATE), so the schedule is
    built around keeping it 100% busy:
    * tiny first ramp block (512 cols) + split weight DMA (edge weights
      are only 33KB) -> first silu at ~4us instead of ~10us
    * ALL DMAs ride the sync ring (the scalar queue is left to ACTIVATEs
      only; its ACT_TABLE_LOAD at t=0 would also serialize any DMA issue
      behind it); ring FIFO order = emission order gives deterministic
      delivery ordering
    * big tables (c4e, nTa, nTb, c4n) are emitted between LATE steady
      edge blocks, where the delivery-vs-consumption slack has built up
    * the c4e-dependent per-block reductions are DEFERRED until c4e has
      safely landed, so the strict-FIFO DVE queue never holds up the
      pe_sb PSUM drains (which gate PE's single pe PSUM bank, which
      gates mm1, which gates ACT)
  - Feature streams are fp16 (halves HBM traffic; verified error budget
    vs the 2e-2 gate on this exact problem instance).
"""

import numpy as np

NCORES = 8
N_NODES = 50000
NUM_GRAPHS = 32
NZ = 101            # atomic number table entries (0..100)
D_EDGE = 128
D_NODE = 256
EDGE_BLOCK = 6144   # steady-state edge columns per DMA block (12KB packets)
SUPER = 1536        # act supertile (3 PSUM banks)
ALIGN = 512         # ET alignment (matches mm1 chunk size)

RAMP = (1024, 2560, 2560, 3072, 3072, 3072, 3072)  # escalating ramp (18432)
# table DMAs are emitted just after these blocks' first units (ring order):
TRIG_C4E_BLK = 9
TRIG_NTA_BLK = 11
TRIG_NTB_BLK = 13
TRIG_C4N_BLK = 14
NODE_START_BLK = 14   # node pairs interleave from this block's first unit
TT_SAFE_BLK = 10      # block reductions flush from this block's first unit

_CACHE = {}


def _edge_blocks(ET):
    """(start, ncols) DMA blocks: escalating ramp then full-width blocks."""
    blocks = []
    pos = 0
    for n in RAMP:
        blocks.append((pos, n))
        pos += n
    while ET - pos >= EDGE_BLOCK:
        blocks.append((pos, EDGE_BLOCK))
        pos += EDGE_BLOCK
    if ET - pos:
        blocks.append((pos, ET - pos))
        pos += ET - pos
    assert pos == ET
    return blocks


def _node_unit_chunks(NT):
    """Greedy (pos, ncols<=512) chunk groups, each group <= SUPER wide."""
    chunks = []
    pos = 0
    while pos < NT:
        c = min(512, NT - pos)
        chunks.append((pos, c))
        pos += c
    units = []
    cur, w = [], 0
    for pos, c in chunks:
        if w + c > SUPER:
            units.append(cur)
            cur, w = [], 0
        cur.append((pos, c))
        w += c
    if cur:
        units.append(cur)
    return units


def _build(ET, NT):
    import concourse.tile as tile
    from concourse import bacc, mybir
    from contextlib import ExitStack

    f32 = mybir.dt.float32
    f16 = mybir.dt.float16
    AF = mybir.ActivationFunctionType
    OP = mybir.AluOpType

    EC = ET // 128
    NTC = NT // 128
    blocks = _edge_blocks(ET)
    NBLK = len(blocks)
    node_units_chunks = _node_unit_chunks(NT)

    nc = bacc.Bacc("TRN2", debug=False, num_devices=NCORES)

    # eT is prefixed with the 129 We columns (W1e | W2e) so the first
    # ramp DMA delivers weights and data in one shot
    eT_d = nc.declare_dram_parameter("eT", [128, 129 + ET], f16, isOutput=False)
    c4e_d = nc.declare_dram_parameter("c4e", [128, 4 * EC], f32, isOutput=False)
    nTa_d = nc.declare_dram_parameter("nTa", [128, NT], f16, isOutput=False)
    nTb_d = nc.declare_dram_parameter("nTb", [128, NT], f16, isOutput=False)
    c4n_d = nc.declare_dram_parameter("c4n", [128, 4 * NTC], f32, isOutput=False)
    # node weights: W1n q00 q01 q10 q11 | W2n  (fp16, 514 cols)
    Wn_d = nc.declare_dram_parameter("Wn", [128, 514], f16, isOutput=False)
    # packed biases: b1e | b1n0 | b1n1  (fp32)
    bp_d = nc.declare_dram_parameter("bp", [128, 3], f32, isOutput=False)
    out1_d = nc.declare_dram_parameter("out1", [128, 4], f32, isOutput=True)
    LBC = blocks[-1][1] // 128
    out2_d = nc.declare_dram_parameter("out2", [128, LBC], f32, isOutput=True)

    with tile.TileContext(nc) as tc, ExitStack() as ctx:
        const = ctx.enter_context(tc.tile_pool(name="const", bufs=1))
        edgep = ctx.enter_context(tc.tile_pool(name="edgep", bufs=5))
        hep = ctx.enter_context(tc.tile_pool(name="hep", bufs=5))
        dvep = ctx.enter_context(tc.tile_pool(name="dvep", bufs=3))
        ps_big = ctx.enter_context(tc.tile_pool(name="ps_big", bufs=2, space="PSUM"))
        ps_pe = ctx.enter_context(tc.tile_pool(name="ps_pe", bufs=1, space="PSUM"))
        ps_acc = ctx.enter_context(tc.tile_pool(name="ps_acc", bufs=1, space="PSUM"))

        # ---- PE warm-up / heartbeat setup ----------------------------------
        # the PE starts HAM-throttled (K=4/8, 1.2 GHz), re-warms only after
        # ~3.4us of sustained activity, and re-throttles whenever it idles
        # too long.  Warm it with garbage MMs during the DMA-latency shadow
        # and keep it warm with one tiny heartbeat MM per unit, all writing
        # spare columns of the persistent pa PSUM bank from a resident
        # (DMA-independent) source tile.
        warm = const.tile([128, 640], f16, name="warmsrc")
        nc.vector.memset(warm[:], 1.0)

        # ---- startup DMAs ---------------------------------------------------
        # ALL DMAs ride the single sync ring: the two HWDGE rings do NOT
        # share SDMA bandwidth fairly (one can starve the other for
        # microseconds), so a strict single-FIFO whose escalating sizes
        # match the serial landing cadence is both faster and predictable
        ramp_xe = []
        r0t = const.tile([128, 129 + RAMP[0]], f16, tag="rampxe0", name="rampxe0")
        nc.sync.dma_start(r0t[:], eT_d.ap()[:, 0:129 + RAMP[0]])
        ramp_xe.append(r0t[:, 129:129 + RAMP[0]])
        We = r0t[:, 0:129]
        bp = const.tile([128, 3], f32)
        nc.sync.dma_start(bp[:], bp_d.ap())
        for r in range(1, len(RAMP)):
            t = const.tile([128, RAMP[r]], f16, tag=f"rampxe{r}", name=f"rampxe{r}")
            pos0 = sum(RAMP[:r])
            nc.sync.dma_start(t[:], eT_d.ap()[:, 129 + pos0:129 + pos0 + RAMP[r]])
            ramp_xe.append(t)
        Wn = const.tile([128, 514], f16)
        nc.sync.dma_start(Wn[:], Wn_d.ap())

        W1e = We[:, 0:128]
        W2e = We[:, 128:129]
        W1n = [Wn[:, q * 128:(q + 1) * 128] for q in range(4)]  # kb*2+db
        W2n = Wn[:, 512:514]
        b1e = bp[:, 0:1]
        b1n = bp[:, 1:3]

        c4e_s = const.tile([128, 4, EC], f32)
        c4n_s = const.tile([128, 4, NTC], f32)
        nTa_s = const.tile([128, NT], f16)
        nTb_s = const.tile([128, NT], f16)
        ones = const.tile([128, 1], f32)
        nc.vector.memset(ones[:], 1.0)

        redsb = const.tile([128, 4, NBLK], f32)
        pesb_all = const.tile([128, NBLK, EDGE_BLOCK // 128], f32)
        pa_all = const.tile([128, NTC], f32)
        pa_ps = ps_acc.tile([128, 512], f32, tag="pa")

        def trig_c4e():
            nc.sync.dma_start(
                c4e_s[:], c4e_d.ap().rearrange("p (g j) -> p g j", g=4))

        def trig_nta():
            nc.sync.dma_start(nTa_s[:], nTa_d.ap())

        def trig_ntb():
            nc.sync.dma_start(nTb_s[:], nTb_d.ap())

        def trig_c4n():
            nc.sync.dma_start(
                c4n_s[:], c4n_d.ap().rearrange("p (g j) -> p g j", g=4))

        # ---- supertile units -----------------------------------------------
        class EdgeState:
            xe = None
            pe_ps = None

        est = [EdgeState() for _ in range(NBLK)]
        deferred = []  # (flush_at_emit_idx, fn): c4e-dependent block reduces

        def make_block_reduce(b):
            pos, ncols = blocks[b]
            cpb = ncols // 128

            def tt():
                jc = slice(pos // 128, pos // 128 + cpb)
                tmpb = dvep.tile(
                    [128, 4, EDGE_BLOCK // 128], f32, tag="tmpb")
                nc.vector.tensor_tensor(
                    tmpb[:, :, 0:cpb], c4e_s[:, :, jc],
                    pesb_all[:, b, 0:cpb].unsqueeze(1).broadcast_to(
                        [128, 4, cpb]),
                    OP.mult,
                )
                nc.vector.tensor_reduce(
                    redsb[:, :, b:b + 1], tmpb[:, :, 0:cpb],
                    mybir.AxisListType.X, OP.add,
                )

            return tt

        def make_edge_unit(b, h):
            pos, ncols = blocks[b]
            hc = min(SUPER, ncols - h * SUPER)
            n512 = -(-hc // 512)
            first = h == 0
            last = (h + 1) * SUPER >= ncols
            cpb = ncols // 128

            def mm1():
                if first:
                    if b < len(RAMP):
                        est[b].xe = ramp_xe[b]
                    else:
                        est[b].xe = edgep.tile(
                            [128, EDGE_BLOCK], f16, tag="xe", name="xe")
                        nc.sync.dma_start(
                            est[b].xe[:, 0:ncols],
                            eT_d.ap()[:, 129 + pos:129 + pos + ncols])
                    est[b].pe_ps = ps_pe.tile(
                        [128, EDGE_BLOCK // 128], f32, tag="pe", name="pe_ps")
                ps = ps_big.tile([128, SUPER], f32, tag="mm1")
                for q in range(n512):
                    qc = min(512, hc - q * 512)
                    nc.tensor.matmul(
                        ps[:, q * 512:q * 512 + qc], W1e,
                        est[b].xe[:, h * SUPER + q * 512:h * SUPER + q * 512 + qc],
                        start=True, stop=True,
                    )
                return ps

            def act(ps):
                he = hep.tile([128, SUPER], f16, tag="he")
                nc.scalar.activation(
                    he[:, 0:hc], ps[:, 0:hc], AF.Silu, bias=b1e)
                return he

            def mm2(he, emit_idx, tt_safe):
                for t in range(hc // 128):
                    col = h * (SUPER // 128) + t
                    nc.tensor.matmul(
                        est[b].pe_ps[:, col:col + 1],
                        he[:, t * 128:(t + 1) * 128], W2e,
                        start=True, stop=True,
                    )
                if last:
                    # prompt PSUM drain (frees the single pe bank for the
                    # next block); the c4e-dependent reduce is deferred so
                    # it never blocks this copy in the FIFO DVE queue
                    nc.vector.tensor_copy(
                        pesb_all[:, b, 0:cpb], est[b].pe_ps[:, 0:cpb])
                    if b < NBLK - 1:
                        deferred.append(
                            (max(emit_idx + 1, tt_safe), make_block_reduce(b)))

            return mm1, act, mm2

        def make_node_pair(chunks):
            """Two act units (K-halves db=0/1) covering the given chunk
            list.  The second unit drains both halves' mm2 back-to-back so
            each pa column's PSUM accumulation group is contiguous."""
            width = sum(c for _, c in chunks)
            j0 = chunks[0][0]
            stash = {}

            def mk(db):
                def mm1():
                    ps = ps_big.tile([128, SUPER], f32, tag="mm1")
                    off = 0
                    for pos, c in chunks:
                        nc.tensor.matmul(
                            ps[:, off:off + c], W1n[0 * 2 + db],
                            nTa_s[:, pos:pos + c],
                            start=True, stop=False,
                        )
                        nc.tensor.matmul(
                            ps[:, off:off + c], W1n[1 * 2 + db],
                            nTb_s[:, pos:pos + c],
                            start=False, stop=True,
                        )
                        off += c
                    return ps

                def act(ps):
                    he = hep.tile([128, SUPER], f16, tag="he")
                    nc.scalar.activation(
                        he[:, 0:width], ps[:, 0:width], AF.Silu,
                        bias=b1n[:, db:db + 1])
                    return he

                def mm2(he, emit_idx, tt_safe):
                    if db == 0:
                        stash["he0"] = he
                        return
                    for t in range(width // 128):
                        col = j0 // 128 + t
                        nc.tensor.matmul(
                            pa_ps[:, col:col + 1],
                            stash["he0"][:, t * 128:(t + 1) * 128],
                            W2n[:, 0:1], start=True, stop=False,
                        )
                        nc.tensor.matmul(
                            pa_ps[:, col:col + 1],
                            he[:, t * 128:(t + 1) * 128],
                            W2n[:, 1:2], start=False, stop=True,
                        )

                return mm1, act, mm2

            return mk(0), mk(1)

        # edge units tagged with their block index
        units = []  # (block_or_None, is_last_node, (mm1, act, mm2))
        for b in range(NBLK):
            _, ncols = blocks[b]
            for h in range(-(-ncols // SUPER)):
                units.append((b, False, make_edge_unit(b, h)))

        def first_unit_idx(blk):
            for i, (b, _, _) in enumerate(units):
                if b == blk:
                    return i
            return len(units) - 1

        # node pairs interleaved from NODE_START_BLK's first unit
        pos_u = first_unit_idx(min(NODE_START_BLK, NBLK - 1))
        node_pairs = [make_node_pair(ch) for ch in node_units_chunks]
        for pi, (ua, ub) in enumerate(node_pairs):
            last = pi == len(node_pairs) - 1
            if pos_u + 1 < len(units):
                units.insert(pos_u, (None, False, ua))
                units.insert(pos_u + 1, (None, last, ub))
                pos_u += 5
            else:
                units.append((None, False, ua))
                units.append((None, last, ub))

        node_last_pos = max(
            i for i, (_, is_last, _) in enumerate(units) if is_last)

        triggers = {}
        for blk, fn in ((TRIG_C4E_BLK, trig_c4e), (TRIG_NTA_BLK, trig_nta),
                        (TRIG_NTB_BLK, trig_ntb), (TRIG_C4N_BLK, trig_c4n)):
            idx = first_unit_idx(min(blk, NBLK - 1))
            while idx in triggers:
                idx += 1
            triggers[idx] = fn
        tt_safe = first_unit_idx(min(TT_SAFE_BLK, NBLK - 1)) + 2

        # node-side final reduction, emitted right after the last node unit
        redn = const.tile([128, 4], f32)

        def node_tail():
            nc.vector.tensor_copy(pa_all[:], pa_ps[:, 0:NTC])
            tmpn = const.tile([128, 4, NTC], f32)
            nc.vector.tensor_tensor(
                tmpn[:], c4n_s[:],
                pa_all[:].unsqueeze(1).broadcast_to([128, 4, NTC]), OP.mult,
            )
            nc.vector.tensor_reduce(
                redn[:].unsqueeze(2), tmpn[:], mybir.AxisListType.X, OP.add,
            )

        # ---- software-pipelined emission -----------------------------------
        red = const.tile([128, 4], f32)  # per-partition per-graph partials
        pending = None  # (mm2, he, emit_idx) of previous unit
        node_tail_done = False
        red_done = False

        def emit_out1():
            # everything except the final block (whose 1-unit sliver is
            # reduced on the host from out2): emitted during the last act
            rede_p = const.tile([128, 4], f32)
            nc.vector.tensor_reduce(
                rede_p[:].unsqueeze(2), redsb[:, :, 0:NBLK - 1],
                mybir.AxisListType.X, OP.add,
            )
            nc.vector.tensor_tensor(red[:], rede_p[:], redn[:], OP.add)
            nc.sync.dma_start(out1_d.ap(), red[:])

        for i, (_, _, (mm1, act, mm2)) in enumerate(units):
            ps = mm1()
            he = act(ps)
            if pending is not None:
                pending[0](pending[1], pending[2], tt_safe)
            pending = (mm2, he, i)
            if i in triggers:
                triggers[i]()
            while deferred and deferred[0][0] <= i:
                deferred.pop(0)[1]()
            if i == node_last_pos + 1:
                node_tail()
                node_tail_done = True
            if (i == len(units) - 1 and len(deferred) == 0
                    and node_tail_done):
                emit_out1()
                red_done = True
        pending[0](pending[1], pending[2], tt_safe)
        for _, fn in deferred:
            fn()
        if not node_tail_done:
            node_tail()
        if not red_done:
            emit_out1()

        # last block's raw per-edge energies; host applies c4e and reduces
        nc.sync.dma_start(out2_d.ap(), pesb_all[:, NBLK - 1, 0:LBC])

    nc.compile()
    return nc


def _shard(inputs):
    node_feats = np.asarray(inputs["node_feats"], dtype=np.float32)
    edge_feats = np.asarray(inputs["edge_feats"], dtype=np.float32)
    Z = np.asarray(inputs["atomic_numbers"], dtype=np.int64)
    idx_s = np.asarray(inputs["idx_s"], dtype=np.int64)
    idx_t = np.asarray(inputs["idx_t"], dtype=np.int64)
    batch = np.asarray(inputs["batch"], dtype=np.int64)
    asc = np.asarray(inputs["atom_scales"], np.float32)[:, 0]
    ash = np.asarray(inputs["atom_shifts"], np.float32)[:, 0]
    pscale = np.asarray(inputs["pair_scales"], np.float32)[:, 0]
    b2e = float(np.asarray(inputs["b2e"], np.float32).reshape(-1)[0])
    b2n = float(np.asarray(inputs["b2n"], np.float32).reshape(-1)[0])

    bounds = np.searchsorted(batch, np.arange(NUM_GRAPHS + 1))
    g_t = batch[idx_t]
    core_of_edge = np.minimum(g_t // 4, NCORES - 1)
    c_e = (pscale[Z[idx_s] * NZ + Z[idx_t]] * asc[Z[idx_t]]).astype(np.float32)

    e_counts = np.bincount(core_of_edge, minlength=NCORES)
    ET = int(-(-e_counts.max() // ALIGN) * ALIGN)
    # keep at least one full-width block after the ramp
    ET = max(ET, sum(RAMP) + EDGE_BLOCK)
    n_counts = bounds[4 * np.arange(NCORES) + 4] - bounds[4 * np.arange(NCORES)]
    NT = int(-(-n_counts.max() // 128) * 128)

    # constant (device-independent) per-graph terms
    asc_n = asc[Z]
    host_add = np.zeros(NUM_GRAPHS, np.float64)
    np.add.at(host_add, batch, (b2n * asc_n + ash[Z]).astype(np.float64))
    if b2e != 0.0:
        np.add.at(host_add, g_t, (b2e * c_e).astype(np.float64))

    order = np.argsort(core_of_edge, kind="stable")
    starts = np.concatenate([[0], np.cumsum(e_counts)])

    W1e = np.asarray(inputs["W1e"], np.float16)
    W1n = np.asarray(inputs["W1n"], np.float16)
    W2e = np.asarray(inputs["W2e"], np.float16).reshape(128, 1)
    W2n = np.asarray(inputs["W2n"], np.float16).reshape(2, 128).T
    We = np.zeros((128, 129), np.float16)
    We[:, 0:128] = W1e
    We[:, 128:129] = W2e
    Wn = np.zeros((128, 514), np.float16)
    for kb in range(2):
        for db in range(2):
            q = kb * 2 + db
            Wn[:, q * 128:(q + 1) * 128] = \
                W1n[kb * 128:(kb + 1) * 128, db * 128:(db + 1) * 128]
    Wn[:, 512:514] = W2n
    bp = np.zeros((128, 3), np.float32)
    bp[:, 0] = np.asarray(inputs["b1e"], np.float32)
    bp[:, 1:3] = np.asarray(inputs["b1n"], np.float32).reshape(2, 128).T

    blocks = _edge_blocks(ET)
    lb_pos, lb_ncols = blocks[-1]

    in_maps = []
    c4e_last = []
    for k in range(NCORES):
        sel = order[starts[k]:starts[k + 1]]
        E = sel.size
        eTk = np.zeros((128, 129 + ET), np.float16)
        eTk[:, 0:129] = We
        eTk[:, 129:129 + E] = edge_feats[sel].T
        c4e = np.zeros((ET, 4), np.float32)
        gl = g_t[sel] - 4 * k
        c4e[np.arange(E), gl] = c_e[sel]
        # last block's [ncols, 4] slice stays on the host: its per-edge
        # energies come back raw in out2 and are reduced here (keeps the
        # final-block reduction off the device's serial tail)
        c4e_last.append(c4e[lb_pos:lb_pos + lb_ncols].copy())
        # [ET,4] -> [128, 4, EC] with edge (j*128+p) at [p, :, j]
        c4e = np.ascontiguousarray(
            c4e.reshape(ET // 128, 128, 4).transpose(1, 2, 0)
        ).reshape(128, -1)

        n0 = int(bounds[4 * k])
        n1 = int(bounds[4 * k + 4])
        nn = n1 - n0
        nTk = np.zeros((256, NT), np.float16)
        nTk[:, :nn] = node_feats[n0:n1].T
        c4n = np.zeros((NT, 4), np.float32)
        c4n[np.arange(nn), batch[n0:n1] - 4 * k] = asc_n[n0:n1]
        c4n = np.ascontiguousarray(
            c4n.reshape(NT // 128, 128, 4).transpose(1, 2, 0)
        ).reshape(128, -1)

        in_maps.append({
            "eT": eTk, "c4e": c4e,
            "nTa": np.ascontiguousarray(nTk[:128]),
            "nTb": np.ascontiguousarray(nTk[128:]),
            "c4n": c4n,
            "Wn": Wn, "bp": bp,
        })
    return ET, NT, in_maps, host_add, c4e_last


LAST_RES = None
LAST_RES_NODE = None


def kernel(**inputs) -> np.ndarray:
    global LAST_RES
    from concourse.bass_utils import run_bass_kernel_spmd

    ET, NT, in_maps, host_add, c4e_last = _shard(inputs)
    key = (ET, NT)
    if key not in _CACHE:
        _CACHE[key] = _build(ET, NT)
    nc = _CACHE[key]

    res = run_bass_kernel_spmd(nc, in_maps, core_ids=list(range(NCORES)))
    LAST_RES = res
    Y = np.zeros(NUM_GRAPHS, np.float64)
    for k in range(NCORES):
        out1 = np.asarray(res.results[k]["out1"], np.float32)  # [128, 4]
        out2 = np.asarray(res.results[k]["out2"], np.float32)  # [128, LBC]
        Y[4 * k:4 * k + 4] += out1.sum(axis=0)
        # out2[p, col] = pe of edge (lb_pos + col*128 + p)
        pe_last = out2.T.reshape(-1)          # [ncols] in edge order
        Y[4 * k:4 * k + 4] += pe_last @ c4e_last[k]
    Y += host_add
    return Y.astype(np.float32)
